# revision 2
# baseline (speedup 1.0000x reference)
"""Trainium2 Bass kernel for nn_Linear_85959475462525.

Reference computation:
    result = (x @ W^T + b) * ood0 + (x @ delta_w^T) * ood1
    delta_w = scatter(spectrum) at (row_idx, col_idx)
    o_loss  = sum((o_spectra @ spectrum)^2)

Algebraic fold (host side, cheap):
    W_eff  = ood0 * W;  W_eff[row_idx, col_idx] = ood0*W[r,c] + ood1*spectrum
    b_eff  = ood0 * b
    result = x @ W_eff^T + b_eff          <- single dense matmul on device
    o_loss computed on host (2k FLOPs).

Device strategy: data-parallel over the 8192 token rows across 8 NeuronCores
(1024 tokens/core). Weight (bf16, 32 MiB) replicated. Each core runs a tiled
bf16 matmul (K=4096 contraction, M=1024 tokens on PSUM partitions, N=4096 out
features) with fp32 PSUM accumulation; the per-out-feature bias is added on
the Vector engine during PSUM->SBUF eviction (replaces the copy, zero cost).
"""

import numpy as np
import ml_dtypes

D_IN = 4096
D_OUT = 4096
N_CORES = 8
TOKENS = 8192
TOK_PER_CORE = TOKENS // N_CORES  # 1024
P = 128

_PROG = None  # cached compiled Bass program


def _build_program():
    import concourse.mybir as mybir
    import concourse.tile as tile
    from concourse import bacc
    from concourse.kernels.tile_matmul import (
        composable_matmul_tile_kernel,
        dma_from_dram_kxm,
        dma_from_dram_kxn,
        dma_to_dram_mxn,
        k_pool_min_bufs,
    )

    nc = bacc.Bacc(
        "TRN2",
        target_bir_lowering=False,
        debug=False,
        num_devices=N_CORES,
    )

    xt = nc.dram_tensor(
        "xt", [D_IN, TOK_PER_CORE], mybir.dt.bfloat16, kind="ExternalInput"
    )
    wt = nc.dram_tensor("wt", [D_IN, D_OUT], mybir.dt.bfloat16, kind="ExternalInput")
    bias = nc.dram_tensor("bias", [P, D_OUT], mybir.dt.float32, kind="ExternalInput")
    out = nc.dram_tensor(
        "out", [TOK_PER_CORE, D_OUT], mybir.dt.float32, kind="ExternalOutput"
    )

    with tile.TileContext(nc) as tc:
        with (
            tc.tile_pool(name="const", bufs=1) as const_pool,
            tc.tile_pool(name="warm_psum", bufs=1, space="PSUM") as warm_psum_pool,
            tc.tile_pool(name="kxm_pool", bufs=k_pool_min_bufs(xt.ap())) as kxm_pool,
            tc.tile_pool(name="kxn_pool", bufs=k_pool_min_bufs(wt.ap())) as kxn_pool,
        ):
            bias_sb = const_pool.tile([P, D_OUT], mybir.dt.float32)
            nc.sync.dma_start(out=bias_sb[:], in_=bias.ap())

            # HAM warmup: keep the PE busy during the initial tile DMAs so the
            # clock gate opens (4/8 -> 8/8) before the first real matmul.
            warm_sb = const_pool.tile([P, P], mybir.dt.bfloat16)
            nc.vector.memset(warm_sb[:], 0.0)
            warm_ps = warm_psum_pool.tile([P, P], mybir.dt.float32)
            for _ in range(28):
                nc.tensor.matmul(
                    warm_ps[:], warm_sb[:], warm_sb[:], start=True, stop=True
                )

            kxm_producer, kxm_shape = dma_from_dram_kxm(kxm_pool, xt.ap())
            kxn_producer, kxn_shape = dma_from_dram_kxn(kxn_pool, wt.ap())
            mxn_consumer = dma_to_dram_mxn(out.ap())

            def bias_add_reducer(nc_, psum, sbuf, md):
                # psum: [128, 512] fp32; sbuf: [128, 1, 512]; bias varies on N.
                nc_.vector.tensor_tensor(
                    sbuf[:, 0, :],
                    psum,
                    bias_sb[:, md.n_subtile_slice],
                    mybir.AluOpType.add,
                )

            composable_matmul_tile_kernel(
                tc,
                kxm_shape=kxm_shape,
                kxn_shape=kxn_shape,
                output_type=mybir.dt.float32,
                kxm_producer=kxm_producer,
                kxn_producer=kxn_producer,
                mxn_consumer=mxn_consumer,
                mxn_subtile_reducer=bias_add_reducer,
            )

    nc.compile()
    return nc


def _get_program():
    global _PROG
    if _PROG is None:
        _PROG = _build_program()
    return _PROG


def _prepare_host(inputs):
    x = np.asarray(inputs["x"], dtype=np.float32)
    base_weight = np.asarray(inputs["base_weight"], dtype=np.float32)
    base_bias = np.asarray(inputs["base_bias"], dtype=np.float32)
    spectrum = np.asarray(inputs["spectrum"], dtype=np.float32)
    ood_weight = np.asarray(inputs["ood_weight"], dtype=np.float32)
    o_spectra = np.asarray(inputs["o_spectra"], dtype=np.float32)
    row_idx = np.asarray(inputs["row_idx"])
    col_idx = np.asarray(inputs["col_idx"])

    w0 = np.float32(ood_weight[0])
    w1 = np.float32(ood_weight[1])

    dots = o_spectra @ spectrum
    o_loss = np.float32(np.sum(dots * dots))

    w_eff = base_weight * w0
    # scatter .set semantics: delta[r, c] = spectrum[i] replaces the zero
    w_eff[row_idx, col_idx] = base_weight[row_idx, col_idx] * w0 + spectrum * w1
    bias_eff = base_bias * w0

    bf16 = ml_dtypes.bfloat16
    wt = np.ascontiguousarray(w_eff.T).astype(bf16)  # [D_IN, D_OUT]
    bias_rep = np.ascontiguousarray(
        np.broadcast_to(bias_eff[None, :], (P, D_OUT))
    ).astype(np.float32)

    x2 = x.reshape(TOKENS, D_IN)
    xt_shards = [
        np.ascontiguousarray(x2[s * TOK_PER_CORE : (s + 1) * TOK_PER_CORE].T).astype(
            bf16
        )
        for s in range(N_CORES)
    ]
    return xt_shards, wt, bias_rep, o_loss


def kernel(_trace=False, **inputs):
    from concourse.bass_utils import run_bass_kernel_spmd

    xt_shards, wt, bias_rep, o_loss = _prepare_host(inputs)
    nc = _get_program()

    in_maps = [
        {"xt": xt_shards[s], "wt": wt, "bias": bias_rep} for s in range(N_CORES)
    ]
    res = run_bass_kernel_spmd(
        nc, in_maps, core_ids=list(range(N_CORES)), trace=_trace
    )
    outs = [res.results[s]["out"] for s in range(N_CORES)]
    result = np.concatenate(outs, axis=0).reshape(4, 2048, D_IN)
    if _trace:
        kernel.last_results = res
    return (result, o_loss)


# revision 4
# speedup vs baseline: 1.0031x; 1.0031x over previous
"""Trainium2 Bass kernel for nn_Linear_85959475462525.

Reference computation:
    result = (x @ W^T + b) * ood0 + (x @ delta_w^T) * ood1
    delta_w = scatter(spectrum) at (row_idx, col_idx)
    o_loss  = sum((o_spectra @ spectrum)^2)

Algebraic fold (host side, cheap):
    W_eff  = ood0 * W;  W_eff[row_idx, col_idx] = ood0*W[r,c] + ood1*spectrum
    b_eff  = ood0 * b
    result = x @ W_eff^T + b_eff          <- single dense matmul on device
    o_loss computed on host (2k FLOPs).

Device strategy: data-parallel over the 8192 token rows across 8 NeuronCores
(1024 tokens/core). Weight (bf16, 32 MiB) replicated. Each core runs a tiled
bf16 matmul (K=4096 contraction, M=1024 tokens on PSUM partitions, N=4096 out
features) with fp32 PSUM accumulation; the per-out-feature bias is added on
the Vector engine during PSUM->SBUF eviction (replaces the copy, zero cost).
"""

import numpy as np
import ml_dtypes

D_IN = 4096
D_OUT = 4096
N_CORES = 8
TOKENS = 8192
TOK_PER_CORE = TOKENS // N_CORES  # 1024
P = 128

_PROG = None  # cached compiled Bass program


def _build_program():
    import concourse.mybir as mybir
    import concourse.tile as tile
    from concourse import bacc
    from concourse.kernels.tile_matmul import (
        composable_matmul_tile_kernel,
        dma_from_dram_kxm,
        dma_from_dram_kxn,
        k_pool_min_bufs,
    )

    nc = bacc.Bacc(
        "TRN2",
        target_bir_lowering=False,
        debug=False,
        num_devices=N_CORES,
        enable_partition_id=False,
    )

    xt = nc.dram_tensor(
        "xt", [D_IN, TOK_PER_CORE], mybir.dt.bfloat16, kind="ExternalInput"
    )
    wt = nc.dram_tensor("wt", [D_IN, D_OUT], mybir.dt.bfloat16, kind="ExternalInput")
    bias = nc.dram_tensor("bias", [P, D_OUT], mybir.dt.float32, kind="ExternalInput")
    out = nc.dram_tensor(
        "out", [TOK_PER_CORE, D_OUT], mybir.dt.float32, kind="ExternalOutput"
    )

    with tile.TileContext(nc) as tc:
        with (
            tc.tile_pool(name="const", bufs=1) as const_pool,
            tc.tile_pool(name="warm_psum", bufs=1, space="PSUM") as warm_psum_pool,
            tc.tile_pool(name="kxm_pool", bufs=k_pool_min_bufs(xt.ap())) as kxm_pool,
            tc.tile_pool(name="kxn_pool", bufs=k_pool_min_bufs(wt.ap())) as kxn_pool,
        ):
            bias_sb = const_pool.tile([P, D_OUT], mybir.dt.float32)
            # Scalar engine's DMA queue: keeps the 2 MiB bias load off the
            # sync-engine queue that feeds the first matmul tiles.
            nc.scalar.dma_start(out=bias_sb[:], in_=bias.ap())

            # HAM warmup: keep the PE busy through the preamble + initial tile
            # DMAs so the clock gate opens (4/8 -> 8/8) and stays open until
            # the first real matmul (~12us in).
            warm_sb = const_pool.tile([P, P], mybir.dt.bfloat16)
            nc.vector.memset(warm_sb[:], 0.0)
            warm_ps = warm_psum_pool.tile([P, P], mybir.dt.float32)
            for _ in range(140):
                nc.tensor.matmul(
                    warm_ps[:], warm_sb[:], warm_sb[:], start=True, stop=True
                )

            kxm_producer, kxm_shape = dma_from_dram_kxm(kxm_pool, xt.ap())
            kxn_producer, kxn_shape = dma_from_dram_kxn(kxn_pool, wt.ap())

            out_t = out.ap().rearrange("(po pi) f -> pi po f", pi=P)

            def bias_add_reducer(nc_, psum, sbuf, md):
                # psum: [128, 512] fp32; sbuf: [128, 1, 512]; bias varies on N.
                # Fuse: PSUM evict = add bias, then stream this subtile out
                # immediately (smaller DMAs -> shorter kernel tail).
                sl = md.n_subtile_slice
                nc_.vector.tensor_tensor(
                    sbuf[:, 0, :],
                    psum,
                    bias_sb[:, sl],
                    mybir.AluOpType.add,
                )
                nc_.sync.dma_start(
                    out_t[:, md.m_tile_idx * md.m_subtiles + md.m_subtile_idx, sl],
                    sbuf[:, 0, :],
                )

            def mxn_consumer(nc_, mxn_tile, md):
                # outputs already streamed per-subtile in the reducer
                pass

            composable_matmul_tile_kernel(
                tc,
                kxm_shape=kxm_shape,
                kxn_shape=kxn_shape,
                output_type=mybir.dt.float32,
                kxm_producer=kxm_producer,
                kxn_producer=kxn_producer,
                mxn_consumer=mxn_consumer,
                mxn_subtile_reducer=bias_add_reducer,
            )

    nc.compile()
    return nc


def _get_program():
    global _PROG
    if _PROG is None:
        _PROG = _build_program()
    return _PROG


def _prepare_host(inputs):
    x = np.asarray(inputs["x"], dtype=np.float32)
    base_weight = np.asarray(inputs["base_weight"], dtype=np.float32)
    base_bias = np.asarray(inputs["base_bias"], dtype=np.float32)
    spectrum = np.asarray(inputs["spectrum"], dtype=np.float32)
    ood_weight = np.asarray(inputs["ood_weight"], dtype=np.float32)
    o_spectra = np.asarray(inputs["o_spectra"], dtype=np.float32)
    row_idx = np.asarray(inputs["row_idx"])
    col_idx = np.asarray(inputs["col_idx"])

    w0 = np.float32(ood_weight[0])
    w1 = np.float32(ood_weight[1])

    dots = o_spectra @ spectrum
    o_loss = np.float32(np.sum(dots * dots))

    w_eff = base_weight * w0
    # scatter .set semantics: delta[r, c] = spectrum[i] replaces the zero
    w_eff[row_idx, col_idx] = base_weight[row_idx, col_idx] * w0 + spectrum * w1
    bias_eff = base_bias * w0

    bf16 = ml_dtypes.bfloat16
    wt = np.ascontiguousarray(w_eff.T).astype(bf16)  # [D_IN, D_OUT]
    bias_rep = np.ascontiguousarray(
        np.broadcast_to(bias_eff[None, :], (P, D_OUT))
    ).astype(np.float32)

    x2 = x.reshape(TOKENS, D_IN)
    xt_shards = [
        np.ascontiguousarray(x2[s * TOK_PER_CORE : (s + 1) * TOK_PER_CORE].T).astype(
            bf16
        )
        for s in range(N_CORES)
    ]
    return xt_shards, wt, bias_rep, o_loss


def kernel(_trace=False, **inputs):
    from concourse.bass_utils import run_bass_kernel_spmd

    xt_shards, wt, bias_rep, o_loss = _prepare_host(inputs)
    nc = _get_program()

    in_maps = [
        {"xt": xt_shards[s], "wt": wt, "bias": bias_rep} for s in range(N_CORES)
    ]
    res = run_bass_kernel_spmd(
        nc, in_maps, core_ids=list(range(N_CORES)), trace=_trace
    )
    outs = [res.results[s]["out"] for s in range(N_CORES)]
    result = np.concatenate(outs, axis=0).reshape(4, 2048, D_IN)
    if _trace:
        kernel.last_results = res
    return (result, o_loss)
